# revision 19
# baseline (speedup 1.0000x reference)
"""Trainium2 Bass kernel for BinaryTokenClassificationModel (segment_reduce).

Reference semantics (B=16, L=2048, H=1024, W=1024):
    src = segment_mean(hidden, source_word_ids)   # [B,W,H]
    tgt = segment_mean(hidden, target_word_ids)   # [B,W,H]
    logits[b,s,t,0] = src[b,s]@w_s + tgt[b,t]@w_t + bias

Linear classifier => project tokens to scalars first:
    dot[l, side] = hidden[l] @ w_side            (PE matmul, H on partitions)
then segment-reduce the scalars via one-hot matmuls (factorize word id as
w = 128*q + r), and emit the [W, W] output as a broadcast outer sum.

Differences vs the previous version: hidden is transposed on the HOST to
[H, L] so the per-token dots run on the tensor engine (w[128h,2].T @
hidT[128h,512l] accumulated over 8 h-chunks) instead of costing ~60us of
DVE+ACT elementwise work; the [2, L] dot rows are transposed back to
token-on-partition layout with 16 tiny PE transposes; one-hots and the
output tiles are bf16 (output upcast to fp32 on the host).

Sharding: data-parallel over batch - 2 examples per NeuronCore on 8 cores.
"""

from contextlib import ExitStack

import ml_dtypes
import numpy as np

import concourse.mybir as mybir
import concourse.tile as tile
from concourse import bacc
from concourse.bass_utils import run_bass_kernel_spmd
from concourse.masks import make_identity

P = 128          # partitions
B = 16           # full batch
NCORES = 8
BLOC = B // NCORES   # batches per core = 2
L = 2048         # tokens
H = 1024         # hidden
W = 1024         # words
Q = W // P       # 8 word chunks (w = q*128 + r)
HC = H // P      # 8 hidden chunks
NI = L // P      # 16 token tiles per batch (token l = i*128 + p)
LT = 4           # dots L-tiles of 512 (PSUM bank size)
LTS = L // LT    # 512

F32 = mybir.dt.float32
BF16 = mybir.dt.bfloat16
I32 = mybir.dt.int32

_CACHE = {}


def _build_module():
    nc = bacc.Bacc(None, target_bir_lowering=False, debug=False)
    names = {}
    with tile.TileContext(nc) as tc, ExitStack() as ctx:
        dram = ctx.enter_context(tc.tile_pool(name="dram", bufs=1, space="DRAM"))
        sb_c = ctx.enter_context(tc.tile_pool(name="const", bufs=1))
        sb_h = ctx.enter_context(tc.tile_pool(name="hid", bufs=8))
        sb_s = ctx.enter_context(tc.tile_pool(name="small", bufs=2))
        sb_o = ctx.enter_context(tc.tile_pool(name="outp", bufs=2))
        ps = ctx.enter_context(tc.tile_pool(name="psum", bufs=1, space="PSUM"))

        # hidden host layout: [P, LT, HC, LTS] per batch -- partition-major so
        # each quarter DMA is one contiguous 8 KB run per partition
        hid_d = [dram.tile([P, LT, HC, LTS], BF16, kind="ExternalInput",
                           name=f"hid{b}") for b in range(BLOC)]
        ids_d = [dram.tile([P, 2, NI], I32, kind="ExternalInput", name=f"ids{b}")
                 for b in range(BLOC)]
        wq_d = dram.tile([P, HC, 2], BF16, kind="ExternalInput")
        const_d = dram.tile([P, NI * P + NI * Q + Q * P], BF16,
                            kind="ExternalInput")
        b_d = dram.tile([P, 1], F32, kind="ExternalInput")
        out_d = [dram.tile([W, W], BF16, kind="ExternalOutput", name=f"logits{b}")
                 for b in range(BLOC)]

        names["hid"] = [t.name for t in hid_d]
        names["ids"] = [t.name for t in ids_d]
        names["w"] = wq_d.name
        names["const"] = const_d.name
        names["b"] = b_d.name
        names["out"] = [t.name for t in out_d]

        # ---- input DMAs first (weights, ids, then quarter-major hidden) ----
        wq_sb = sb_c.tile([P, HC, 2], BF16, tag="wq")
        nc.sync.dma_start(out=wq_sb[:], in_=wq_d[:])
        const_sb = sb_c.tile([P, NI * P + NI * Q + Q * P], BF16, tag="cst")
        nc.sync.dma_start(out=const_sb[:], in_=const_d[:])
        iota_r16 = const_sb[:, 0:NI * P].rearrange("p (i r) -> p i r", i=NI)
        iota_q16 = const_sb[:, NI * P:NI * P + NI * Q].rearrange(
            "p (i q) -> p i q", i=NI)
        ident_rep = const_sb[:, NI * P + NI * Q:].rearrange(
            "p (q c) -> p q c", q=Q)
        ht_all = [[None] * LT for _ in range(BLOC)]

        def load_quarter(b, q):
            ht = sb_h.tile([P, HC, LTS], BF16, tag="ht", name=f"ht{b}_{q}")
            nc.sync.dma_start(out=ht[:], in_=hid_d[b][:][:, q, :, :])
            ht_all[b][q] = ht

        ids_all = []
        for b in range(BLOC):
            ids_t = sb_s.tile([P, 2, NI], I32, tag="ids", name=f"ids_t{b}")
            nc.sync.dma_start(out=ids_t[:], in_=ids_d[b][:])
            ids_all.append(ids_t)
        for b in range(BLOC):
            for q in range(LT):
                load_quarter(b, q)
        b_sb = sb_c.tile([P, 1], F32, tag="bb")
        nc.sync.dma_start(out=b_sb[:], in_=b_d[:])

        ones_b = sb_c.tile([P, P], BF16, tag="ones")
        nc.vector.memset(ones_b[:], 1.0)

        # ---- phase 1: ids -> one-hots for both batches (overlaps loads) ----
        or_all_b, mdoq_b = [], []
        for b in range(BLOC):
            ids_t = ids_all[b]
            q_i = sb_s.tile([P, 2, NI], I32, tag="qi")
            r_i = sb_s.tile([P, 2, NI], I32, tag="ri")
            nc.vector.tensor_scalar(out=q_i[:], in0=ids_t[:], scalar1=7,
                                    scalar2=None,
                                    op0=mybir.AluOpType.logical_shift_right)
            nc.vector.tensor_scalar(out=r_i[:], in0=ids_t[:], scalar1=127,
                                    scalar2=None,
                                    op0=mybir.AluOpType.bitwise_and)
            qf = sb_s.tile([P, 2, NI], BF16, tag="qf")
            rf = sb_s.tile([P, 2, NI], BF16, tag="rf")
            nc.vector.tensor_copy(out=qf[:], in_=q_i[:])
            nc.vector.tensor_copy(out=rf[:], in_=r_i[:])
            or_all = sb_s.tile([P, 2, NI, P], BF16, tag="orall",
                               name=f"orall{b}")
            for s in range(2):
                nc.vector.tensor_tensor(
                    out=or_all[:, s, :, :], in0=iota_r16,
                    in1=rf[:, s, :].to_broadcast([P, NI, P]),
                    op=mybir.AluOpType.is_equal)
            mdoq = sb_s.tile([P, 2, NI, 2 * Q], BF16, tag="mdoq",
                             name=f"mdoq{b}")
            for s in range(2):
                nc.vector.tensor_tensor(
                    out=mdoq[:, s, :, Q:2 * Q], in0=iota_q16,
                    in1=qf[:, s, :].to_broadcast([P, NI, Q]),
                    op=mybir.AluOpType.is_equal)
            or_all_b.append(or_all)
            mdoq_b.append(mdoq)

        # ---- phase 2: interleaved dots / post pipelines ----
        dots_row_b = [[None] * LT for _ in range(BLOC)]
        dt_ps_b = [None] * BLOC
        seg_ps_b = [None] * BLOC
        dots_sb_b = [None] * BLOC
        proj_b = [None] * BLOC
        projs_b = [None] * BLOC
        bc_sb_b = [None] * BLOC

        def emit_dots(b, q):
            # dots for the quarter, evacuation copy, and the 4 transposes;
            # all hide under the next quarter's DMA
            ht = ht_all[b][q]
            dots_q = ps.tile([2, LTS], F32, space="PSUM", tag="dots", bufs=2,
                             name=f"dots{b}_{q}")
            for c in range(HC):
                nc.tensor.matmul(out=dots_q[:], lhsT=wq_sb[:, c, :],
                                 rhs=ht[:, c, :],
                                 start=(c == 0), stop=(c == HC - 1))
            dots_row = sb_s.tile([2, LTS], BF16, tag="drow",
                                 name=f"drow{b}_{q}", bufs=8)
            if b == BLOC - 1 and q == LT - 1:
                nc.vector.tensor_copy(out=dots_row[:], in_=dots_q[:])
            else:
                nc.scalar.copy(out=dots_row[:], in_=dots_q[:])
            if dt_ps_b[b] is None:
                dt_ps_b[b] = ps.tile([P, NI, 2], BF16, space="PSUM", tag="dt",
                                     bufs=2, name=f"dt{b}",
                                     padded_shape=[P, NI, 32])
            for k in range(LT):
                i = q * LT + k
                nc.tensor.transpose(out=dt_ps_b[b][:, i, :],
                                    in_=dots_row[:, k * P:(k + 1) * P],
                                    identity=ident_rep[0:2, 0, 0:2])
            dots_row_b[b][q] = dots_row

        def emit_dtmult(b):
            # mdoq[.., 0:Q] = q-onehot * dot, reading the transposed dots
            # straight out of PSUM (broadcast already forces 1x mode)
            mdoq = mdoq_b[b]
            for s in range(2):
                nc.vector.tensor_tensor(
                    out=mdoq[:, s, :, 0:Q], in0=mdoq[:, s, :, Q:2 * Q],
                    in1=dt_ps_b[b][:, :, s].to_broadcast([P, NI, Q]),
                    op=mybir.AluOpType.mult)

        def emit_seg(b):
            # seg_ps[s][r, qc] = sum_{p,i} or_all[p, s, i, r] * mdoq[p, s, i, qc]
            seg_ps = [ps.tile([P, 2 * Q], F32, space="PSUM", tag="segps",
                              bufs=2, name=f"segps{b}_{s}") for s in range(2)]
            for s in range(2):
                for i in range(NI):
                    nc.tensor.matmul(out=seg_ps[s][:],
                                     lhsT=or_all_b[b][:, s, i, :],
                                     rhs=mdoq_b[b][:, s, i, :],
                                     start=(i == 0), stop=(i == NI - 1))
            seg_ps_b[b] = seg_ps

        def emit_epilogue(b):
            seg_ps = seg_ps_b[b]
            cnt = sb_s.tile([P, 2, Q], F32, tag="cnt")
            rec = sb_s.tile([P, 2, Q], F32, tag="rec")
            for s in range(2):
                nc.vector.tensor_scalar(out=cnt[:, s, :],
                                        in0=seg_ps[s][:, Q:2 * Q],
                                        scalar1=1.0, scalar2=None,
                                        op0=mybir.AluOpType.max)
            nc.vector.reciprocal(out=rec[:], in_=cnt[:])
            projt = sb_s.tile([P, Q], BF16, tag="projt", name=f"projt{b}")
            nc.vector.tensor_tensor(out=projt[:], in0=seg_ps[1][:, 0:Q],
                                    in1=rec[:, 1, :], op=mybir.AluOpType.mult)
            projf = sb_s.tile([P, Q], F32, tag="projf", name=f"projf{b}")
            nc.vector.tensor_tensor(out=projf[:], in0=seg_ps[0][:, 0:Q],
                                    in1=rec[:, 0, :], op=mybir.AluOpType.mult)
            projs = sb_s.tile([P, Q], F32, tag="projs", name=f"projs{b}")
            nc.vector.tensor_scalar(out=projs[:], in0=projf[:],
                                    scalar1=b_sb[:, 0:1], scalar2=None,
                                    op0=mybir.AluOpType.add)
            proj_b[b] = projt
            projs_b[b] = projs

        def emit_msel_bc(b):
            projt = proj_b[b]
            msel = sb_s.tile([P, Q, P], BF16, tag="msel")
            nc.vector.tensor_tensor(out=msel[:], in0=ident_rep,
                                    in1=projt[:].to_broadcast([P, Q, P]),
                                    op=mybir.AluOpType.mult)
            bc_sb = sb_s.tile([P, W], BF16, tag="bcsb", name=f"bcsb{b}")
            for half in range(2):
                bc_ps = ps.tile([P, W // 2], F32, space="PSUM", tag="bc",
                                bufs=2, name=f"bc{b}_{half}")
                nc.tensor.matmul(out=bc_ps[:], lhsT=ones_b[:],
                                 rhs=msel[:, half * (Q // 2):(half + 1) * (Q // 2), :],
                                 start=True, stop=True)
                if half == 0:
                    nc.vector.tensor_copy(out=bc_sb[:, 0:W // 2], in_=bc_ps[:])
                else:
                    nc.scalar.copy(out=bc_sb[:, W // 2:W], in_=bc_ps[:])
            bc_sb_b[b] = bc_sb

        def emit_outs(b):
            out_ap = out_d[b][:].rearrange("(j p) t -> p j t", p=P)
            bc_sb, projs = bc_sb_b[b], projs_b[b]
            if b == BLOC - 1:
                for jp in range(4):
                    ot = sb_o.tile([P, 2, W], BF16, tag="ot2", bufs=4)
                    for k in range(2):
                        j = jp * 2 + k
                        nc.vector.tensor_scalar(
                            out=ot[:, k, :], in0=bc_sb[:],
                            scalar1=projs[:, j:j + 1],
                            scalar2=None, op0=mybir.AluOpType.add)
                    dma_eng = nc.sync if jp % 2 == 0 else nc.scalar
                    dma_eng.dma_start(out=out_ap[:, jp * 2:jp * 2 + 2, :],
                                      in_=ot[:])
                return
            eng = ["v", "a", "v", "a", "v", "a", "v", "a"]
            for jp in range(2):
                ot = sb_o.tile([P, 4, W], BF16, tag="ot", bufs=2)
                for k in range(4):
                    j = jp * 4 + k
                    if eng[j] == "v":
                        nc.vector.tensor_scalar(
                            out=ot[:, k, :], in0=bc_sb[:],
                            scalar1=projs[:, j:j + 1],
                            scalar2=None, op0=mybir.AluOpType.add)
                    else:
                        nc.scalar.add(out=ot[:, k, :], in_=bc_sb[:],
                                      add=projs[:, j:j + 1])
                dma_eng = nc.sync if jp % 2 == 0 else nc.scalar
                dma_eng.dma_start(out=out_ap[:, jp * 4:jp * 4 + 4, :], in_=ot[:])

        # interleave: b1's DMA-paced dots fill the PE while b0's post-chain
        # (which ping-pongs PE<->DVE) resolves, and vice versa
        for q in range(LT):
            emit_dots(0, q)
        emit_dots(1, 0)
        emit_dtmult(0)
        emit_dots(1, 1)
        emit_seg(0)
        emit_dots(1, 2)
        emit_epilogue(0)
        emit_msel_bc(0)
        emit_dots(1, 3)
        emit_dtmult(1)
        emit_seg(1)
        emit_epilogue(1)
        emit_msel_bc(1)
        emit_outs(0)
        emit_outs(1)

    nc.compile()
    return nc, names


def _get_module():
    if "mod" not in _CACHE:
        _CACHE["mod"] = _build_module()
    return _CACHE["mod"]


def _run(hidden, classifier_w, classifier_b, source_word_ids, target_word_ids,
         **spmd_kwargs):
    nc, names = _get_module()
    bf16 = ml_dtypes.bfloat16
    hidden = np.asarray(hidden, dtype=np.float32)
    # [B, P, LT, HC, LTS] bf16: hidT[b, p, q, c, n] = hidden[b, q*512+n, c*128+p]
    hidT = np.ascontiguousarray(
        hidden.transpose(0, 2, 1).reshape(B, HC, P, LT, LTS)
        .transpose(0, 2, 3, 1, 4)).astype(bf16)

    w = np.asarray(classifier_w, dtype=np.float32).reshape(2 * H)
    # wq[p, c, s] = w_side_s[c*128 + p]
    wq = np.ascontiguousarray(
        np.stack([w[:H].reshape(HC, P).T, w[H:].reshape(HC, P).T],
                 axis=-1).astype(bf16))
    bias = np.ascontiguousarray(
        np.broadcast_to(np.asarray(classifier_b, dtype=np.float32)
                        .reshape(1, 1), (P, 1)))

    iota_r16 = np.broadcast_to(np.arange(P, dtype=np.float32), (NI, P))
    iota_q16 = np.broadcast_to(np.arange(Q, dtype=np.float32), (NI, Q))
    ident_rep = np.zeros((P, Q, P), dtype=np.float32)
    for p in range(P):
        ident_rep[p, :, p] = 1.0
    cst = np.concatenate([
        np.broadcast_to(iota_r16.reshape(1, -1), (P, NI * P)),
        np.broadcast_to(iota_q16.reshape(1, -1), (P, NI * Q)),
        ident_rep.reshape(P, Q * P)], axis=1).astype(bf16)
    cst = np.ascontiguousarray(cst)

    src = np.asarray(source_word_ids, dtype=np.int32)
    tgt = np.asarray(target_word_ids, dtype=np.int32)
    # idsT[b, p, s, i] = ids_side[b, i*128 + p]
    idsT = np.ascontiguousarray(
        np.stack([src.reshape(B, NI, P).transpose(0, 2, 1),
                  tgt.reshape(B, NI, P).transpose(0, 2, 1)], axis=2))

    in_maps = []
    for c in range(NCORES):
        m = {names["w"]: wq, names["b"]: bias, names["const"]: cst}
        for b in range(BLOC):
            gb = c * BLOC + b
            m[names["hid"][b]] = hidT[gb]
            m[names["ids"][b]] = idsT[gb]
        in_maps.append(m)

    res = run_bass_kernel_spmd(nc, in_maps, core_ids=list(range(NCORES)),
                               **spmd_kwargs)
    out = np.empty((B, W, W, 1), dtype=np.float32)
    for c in range(NCORES):
        for b in range(BLOC):
            out[c * BLOC + b, :, :, 0] = np.asarray(
                res.results[c][names["out"][b]], dtype=np.float32)
    return out, res


def kernel(hidden, classifier_w, classifier_b, source_word_ids,
           target_word_ids, num_words):
    out, _ = _run(hidden, classifier_w, classifier_b, source_word_ids,
                  target_word_ids)
    return out
